# revision 12
# baseline (speedup 1.0000x reference)
"""Multi-head self-attention + LayerNorm, sharded over 8 TRN2 NeuronCores.

Problem: x[4, 2048, 1024], 16 heads x 64 dim, causal attention, output
projection, LayerNorm.  Sharding: core c handles batch c//2 and head-group
c%2 (8 heads).  All 8 cores run one SPMD program; the output projection
produces partial sums which are pair-wise AllReduced on device, then each
core applies the final LayerNorm.  Host gathers batch b from core 2*b.

Matmuls run in float32r (fp32 container, mantissa rounded to 11 explicit
bits) which streams through the PE at full rate (1 cycle/row at N>=256) vs
4 cycles/row for fp32.  The BIR verifier requires every f32r matmul operand
to be produced "rounded": DRAM inputs are pre-rounded on the host and
declared f32r; on-chip operands are produced by ACT/DVE ops with f32r
output dtype (those engines round on write).
"""

import numpy as np

import concourse.bass as bass
import concourse.mybir as mybir
import concourse.tile as tile
from concourse import bacc
from concourse.bass_utils import run_bass_kernel_spmd

# Problem constants (hardcoded per harness contract)
B, T, C = 4, 2048, 1024
H, D = 16, 64
HG = 2                 # head groups (cores per batch)
HPG = H // HG          # heads per group = 8
CG = C // HG           # channels per group = 512
SCALE = D ** -0.5      # 0.125
LN_EPS = 1e-5

QT = 512               # q tile (moving free dim)
KT = 128               # k tile (PE contraction tile)
NQT = T // QT          # 4
NKC = T // KT          # 16
NIC = C // 128         # 8 input-channel chunks
NDC = CG // 128        # 4 output d-chunks per group

F32 = mybir.dt.float32
F32R = mybir.dt.float32r

REPLICA_GROUPS = [[0, 1], [2, 3], [4, 5], [6, 7]]


def build_program():
    """Build + compile the single-core SPMD Bass program. Returns (nc, io)."""
    nc = bacc.Bacc(
        "TRN2",
        target_bir_lowering=False,
        debug=False,
        enable_asserts=False,
        num_devices=8,
    )

    # ---- DRAM I/O ----  (f32r inputs are pre-rounded fp32 on the host)
    xT = nc.dram_tensor("xT", [C, T], F32R, kind="ExternalInput")
    wqT = nc.dram_tensor("wqT", [C, CG], F32R, kind="ExternalInput")
    wkT = nc.dram_tensor("wkT", [C, CG], F32R, kind="ExternalInput")
    wvT = nc.dram_tensor("wvT", [C, CG], F32R, kind="ExternalInput")
    wpT = nc.dram_tensor("wpT", [CG, C], F32R, kind="ExternalInput")
    gamma = nc.dram_tensor("gamma", [C], F32, kind="ExternalInput")
    beta = nc.dram_tensor("beta", [C], F32, kind="ExternalInput")
    # 4 diagonal-block masks [128 k x 512 q]: 1.0 where 128*j + k_r <= q_r
    masks = nc.dram_tensor("masks", [4, KT, QT], F32, kind="ExternalInput")
    y_out = nc.dram_tensor("y", [T, C], F32, kind="ExternalOutput")

    with tile.TileContext(nc) as tc:
        _body(tc, xT, wqT, wkT, wvT, wpT, gamma, beta, masks, y_out)

    nc.compile()
    io = dict(inputs=["xT", "wqT", "wkT", "wvT", "wpT", "gamma", "beta", "masks"],
              output="y")
    return nc, io


def _body(tc, xT, wqT, wkT, wvT, wpT, gamma, beta, masks, y_out):
    nc = tc.nc

    # ---------- persistent SBUF ----------
    persist = tc.alloc_tile_pool(name="persist", bufs=1)
    # K^T / Q^T in [128 part, d-chunk, t] layout; head h lives at partition
    # rows 64*(h%2) .. +64 of chunk h//2.  Q^T is overwritten in-place by the
    # normalized attention output O^T (identical layout), which then feeds the
    # output projection as lhsT.
    kT_sb = persist.tile([128, NDC, T], F32R)
    qT_sb = persist.tile([128, NDC, T], F32R)
    # V in [t(128-chunks) part, k-chunk, head, 65] layout; col 64 is the ones
    # column providing the softmax denominator in the AV matmul.
    v_sb = persist.tile([128, NKC, HPG, 65], F32R)
    mask_sb = persist.tile([128, 4, QT], F32)
    ones_sb = persist.tile([65, 64], F32R)
    eps_sb = persist.tile([128, 1], F32)

    nc.sync.dma_start(out=mask_sb, in_=masks.ap().rearrange("j k q -> k j q"))
    nc.vector.memset(eps_sb, LN_EPS)
    # f32r tiles cannot be memset directly (ISA check); round through a copy
    ones_f = persist.tile([128, 128], F32)
    nc.vector.memset(ones_f, 1.0)
    nc.scalar.copy(ones_sb, ones_f[0:65, 0:64])
    # ones columns of V
    nc.scalar.copy(
        v_sb[:, :, :, 64],
        ones_f[:, 0:NKC * HPG].rearrange("p (a b) -> p a b", a=NKC),
    )

    # ---------- phase A: QKV projections ----------
    with (
        tc.tile_pool(name="wproj", bufs=1) as wpool,
        tc.tile_pool(name="xstream", bufs=2) as xpool,
        tc.tile_pool(name="psA", bufs=3, space="PSUM") as psA,
    ):
        wq_sb = wpool.tile([128, NIC, CG], F32R, tag="wq")
        wk_sb = wpool.tile([128, NIC, CG], F32R, tag="wk")
        wv_sb = wpool.tile([128, NIC, CG], F32R, tag="wv")
        nc.sync.dma_start(out=wq_sb, in_=wqT.ap().rearrange("(a p) o -> p a o", p=128))
        nc.sync.dma_start(out=wk_sb, in_=wkT.ap().rearrange("(a p) o -> p a o", p=128))
        nc.sync.dma_start(out=wv_sb, in_=wvT.ap().rearrange("(a p) o -> p a o", p=128))

        for tt in range(NQT):  # t tiles of 512
            ts = slice(tt * QT, (tt + 1) * QT)
            x_t = xpool.tile([128, NIC, QT], F32R)
            nc.sync.dma_start(out=x_t, in_=xT.ap()[:, ts].rearrange("(a p) t -> p a t", p=128))

            # K^T and Q^T: out[d_chunk, t] = sum_i W[d, i] * xT[i, t]
            for w_sb, dst in ((wk_sb, kT_sb), (wq_sb, qT_sb)):
                for dc in range(NDC):
                    ps = psA.tile([128, QT], F32, tag="ps")
                    for ic in range(NIC):
                        nc.tensor.matmul(
                            ps,
                            w_sb[:, ic, dc * 128:(dc + 1) * 128],
                            x_t[:, ic, :],
                            start=(ic == 0), stop=(ic == NIC - 1),
                        )
                    nc.scalar.copy(dst[:, dc, ts], ps)

            # V: out[t(128) part, d] = sum_i xT[i, t] * WvT[i, d]
            for j in range(QT // 128):
                kc = tt * (QT // 128) + j
                ps = psA.tile([128, CG], F32, tag="ps")
                for ic in range(NIC):
                    nc.tensor.matmul(
                        ps,
                        x_t[:, ic, j * 128:(j + 1) * 128],
                        wv_sb[:, ic, :],
                        start=(ic == 0), stop=(ic == NIC - 1),
                    )
                nc.scalar.copy(
                    v_sb[:, kc, :, 0:64],
                    ps.rearrange("p (h d) -> p h d", h=HPG),
                )

    # ---------- phase B: attention ----------
    with (
        tc.tile_pool(name="psS", bufs=2, space="PSUM") as psS,
        tc.tile_pool(name="psO", bufs=3, space="PSUM") as psO,
        tc.tile_pool(name="psR", bufs=1, space="PSUM") as psR,
        tc.tile_pool(name="pT", bufs=3) as ppool,
        tc.tile_pool(name="norm", bufs=4) as npool,
    ):
        for hp in range(HPG // 2):  # head pairs
            for qt in range(NQT):
                nkt = (qt + 1) * (QT // KT)  # causal k-extent in 128-tiles
                ps_o = [psO.tile([65, QT], F32, tag="pso", name=f"ps_o{e}")
                        for e in range(2)]

                def s_pair(kt):
                    # S^T[k, q] = K[k, :] . Q[q, :]; heads 2hp / 2hp+1 live on
                    # partition rows 0-63 / 64-127 -> row-tiled, run concurrent
                    ps_s = psS.tile([128, 2, QT], F32, tag="pss", name="ps_s")
                    for e in range(2):
                        r0, r1 = 64 * e, 64 * e + 64
                        nc.tensor.matmul(
                            ps_s[:, e, :],
                            kT_sb[r0:r1, hp, kt * KT:(kt + 1) * KT],
                            qT_sb[r0:r1, hp, qt * QT:(qt + 1) * QT],
                            start=True, stop=True,
                        )
                    return ps_s

                # software-pipelined: S-pair for kt+1 issues before AV of kt
                ps_s_cur = s_pair(0)
                for kt in range(nkt):
                    ps_s_next = s_pair(kt + 1) if kt + 1 < nkt else None
                    diag = kt - (nkt - 4)  # >=0 on diagonal blocks
                    p_t = ppool.tile([128, 2, QT], F32R, tag="pt")
                    # exp over both heads' tiles in one ACT call
                    nc.scalar.activation(
                        p_t, ps_s_cur, mybir.ActivationFunctionType.Exp, scale=SCALE,
                    )
                    if diag >= 0:
                        for e in range(2):
                            nc.vector.tensor_mul(
                                p_t[:, e, :], p_t[:, e, :], mask_sb[:, diag, :]
                            )
                    for e in range(2):
                        h = 2 * hp + e
                        nc.tensor.matmul(
                            ps_o[e],
                            v_sb[:, kt, h, :],
                            p_t[:, e, :],
                            start=(kt == 0), stop=(kt == nkt - 1),
                        )
                    ps_s_cur = ps_s_next

                # normalize: O^T[d, q] /= denom[q]; write into qT_sb (=out^T)
                for e in range(2):
                    r_t = npool.tile([65, QT], F32R, tag="recip")
                    with nc.allow_low_precision("f32r reciprocal is 12-bit mantissa"):
                        nc.vector.reciprocal(r_t[64:65, :], ps_o[e][64:65, :])
                    # broadcast denom row across 64 partitions: ones outer-prod
                    rb_ps = psR.tile([64, QT], F32, tag="rbps")
                    nc.tensor.matmul(
                        rb_ps, ones_sb[64:65, :], r_t[64:65, :],
                        start=True, stop=True,
                    )
                    rb = npool.tile([64, QT], F32, tag="rbcast")
                    nc.vector.tensor_copy(rb, rb_ps)
                    if e == 0:
                        nc.vector.tensor_mul(
                            qT_sb[0:64, hp, qt * QT:(qt + 1) * QT],
                            ps_o[e][0:64, :], rb,
                        )
                    else:
                        o_n = npool.tile([64, QT], F32R, tag="onorm")
                        nc.vector.tensor_mul(o_n, ps_o[e][0:64, :], rb)
                        nc.sync.dma_start(
                            out=qT_sb[64:128, hp, qt * QT:(qt + 1) * QT], in_=o_n
                        )

    # ---------- phase C: output projection + pair AllReduce + LayerNorm ----------
    with (
        tc.tile_pool(name="wp", bufs=1) as wppool,
        tc.tile_pool(name="psY", bufs=2, space="PSUM") as psY,
        tc.tile_pool(name="ytile", bufs=3) as ypool,
        tc.tile_pool(name="lnst", bufs=4) as lnpool,
        tc.tile_pool(name="dram", bufs=1, space="DRAM") as dram,
    ):
        wp_sb = wppool.tile([128, NDC, C], F32R)
        nc.sync.dma_start(out=wp_sb, in_=wpT.ap().rearrange("(a p) o -> p a o", p=128))
        gamma_sb = wppool.tile([128, C], F32)
        beta_sb = wppool.tile([128, C], F32)
        nc.gpsimd.dma_start(out=gamma_sb, in_=gamma.ap().unsqueeze(0).to_broadcast([128, C]))
        nc.gpsimd.dma_start(out=beta_sb, in_=beta.ap().unsqueeze(0).to_broadcast([128, C]))

        y_part = dram.tile([T, C], F32)
        y_red = dram.tile([T, C], F32)

        NCHUNK = 2  # collective chunks for overlap
        rows_per_chunk = T // NCHUNK
        for ch in range(NCHUNK):
            for tn in range(ch * (T // 128 // NCHUNK), (ch + 1) * (T // 128 // NCHUNK)):
                y_sb = ypool.tile([128, C], F32, tag="ysb")
                for ct in range(C // QT):
                    ps = psY.tile([128, QT], F32, tag="psy")
                    for cc in range(NDC):
                        nc.tensor.matmul(
                            ps,
                            qT_sb[:, cc, tn * 128:(tn + 1) * 128],
                            wp_sb[:, cc, ct * QT:(ct + 1) * QT],
                            start=(cc == 0), stop=(cc == NDC - 1),
                        )
                    nc.scalar.copy(y_sb[:, ct * QT:(ct + 1) * QT], ps)
                nc.sync.dma_start(out=y_part[tn * 128:(tn + 1) * 128, :], in_=y_sb)
            rs = slice(ch * rows_per_chunk, (ch + 1) * rows_per_chunk)
            nc.gpsimd.collective_compute(
                "AllReduce",
                mybir.AluOpType.add,
                replica_groups=REPLICA_GROUPS,
                ins=[y_part[rs, :]],
                outs=[y_red[rs, :]],
            )

        # LayerNorm over last dim, rows on partitions
        for tn in range(T // 128):
            y_t = ypool.tile([128, C], F32, tag="yln")
            nc.sync.dma_start(out=y_t, in_=y_red[tn * 128:(tn + 1) * 128, :])
            stats = lnpool.tile([128, 2, 6], F32, tag="stats")
            mv = lnpool.tile([128, 2], F32, tag="mv")
            yv = y_t.rearrange("p (s f) -> p s f", s=2)
            for s in range(2):
                nc.vector.bn_stats(out=stats[:, s, :], in_=yv[:, s, :])
            nc.vector.bn_aggr(out=mv, in_=stats)
            rstd = lnpool.tile([128, 1], F32, tag="rstd")
            nc.scalar.activation(
                out=rstd, in_=mv[:, 1:2],
                func=mybir.ActivationFunctionType.Sqrt,
                bias=eps_sb, scale=1.0,
            )
            nc.vector.reciprocal(rstd, rstd)
            nc.vector.tensor_scalar(
                out=y_t, in0=y_t,
                scalar1=mv[:, 0:1], scalar2=rstd,
                op0=mybir.AluOpType.subtract, op1=mybir.AluOpType.mult,
            )
            nc.vector.tensor_mul(y_t, y_t, gamma_sb)
            nc.vector.tensor_add(y_t, y_t, beta_sb)
            nc.sync.dma_start(out=y_out.ap()[tn * 128:(tn + 1) * 128, :], in_=y_t)

    persist.release()


_PROG = None


def _get_program():
    global _PROG
    if _PROG is None:
        _PROG = build_program()
    return _PROG


def _round_f32r(a):
    """Round fp32 to the f32r grid (11 explicit mantissa bits, RNE-ish)."""
    bits = np.ascontiguousarray(a, np.float32).view(np.uint32)
    r = ((bits.astype(np.uint64) + 0x800) & 0xFFFFF000).astype(np.uint32)
    return r.view(np.float32)


def make_in_maps(x, Wk, Wq, Wv, Wp, gamma, beta):
    x = np.asarray(x, dtype=np.float32)
    masks = np.zeros((4, KT, QT), dtype=np.float32)
    for j in range(4):
        k = np.arange(KT)[:, None]
        q = np.arange(QT)[None, :]
        masks[j] = (128 * j + k <= q).astype(np.float32)
    in_maps = []
    for c in range(8):
        b, hg = c // HG, c % HG
        sl = slice(hg * CG, (hg + 1) * CG)
        in_maps.append({
            "xT": _round_f32r(x[b].T),
            "wqT": _round_f32r(np.asarray(Wq, np.float32)[sl, :].T),
            "wkT": _round_f32r(np.asarray(Wk, np.float32)[sl, :].T),
            "wvT": _round_f32r(np.asarray(Wv, np.float32)[sl, :].T),
            "wpT": _round_f32r(np.asarray(Wp, np.float32)[:, sl].T),
            "gamma": np.asarray(gamma, np.float32),
            "beta": np.asarray(beta, np.float32),
            "masks": masks,
        })
    return in_maps


def kernel(x, Wk, Wq, Wv, Wp, gamma, beta, _trace=False, _trace_kwargs=None):
    nc, io = _get_program()
    in_maps = make_in_maps(x, Wk, Wq, Wv, Wp, gamma, beta)
    res = run_bass_kernel_spmd(
        nc, in_maps, core_ids=list(range(8)),
        trace=_trace, **(_trace_kwargs or {}),
    )
    out = np.stack([res.results[HG * b]["y"] for b in range(B)])
    if _trace:
        kernel.last_results = res
    return out


# revision 14
# speedup vs baseline: 1.2623x; 1.2623x over previous
"""Multi-head self-attention + LayerNorm, sharded over 8 TRN2 NeuronCores.

Problem: x[4, 2048, 1024], 16 heads x 64 dim, causal attention, output
projection, LayerNorm.  Sharding: core c handles batch c//2 and head-group
c%2 (8 heads).  All 8 cores run one SPMD program; the output projection
produces partial sums which are pair-wise AllReduced on device, then each
core applies the final LayerNorm.  Host gathers batch b from core 2*b.

Dtypes: projections and QK^T run in float32r (fp32 container, mantissa
rounded to 11 explicit bits; full PE rate at N>=256).  The BIR verifier
requires f32r matmul operands to be produced "rounded": DRAM inputs are
pre-rounded on the host and declared f32r; on-chip operands come from
ACT/DVE ops with f32r output (those engines round on write).  The
AV (softmax-weights x V) matmul runs in bf16 -- P is in [0,1] and V error
averages out, while fp32r's inline 4-byte weight load makes f32r AV ~2.4x
slower than bf16.

Schedule: the attention loop runs q-tile-major; as soon as a 512-row q-tile
has all heads' outputs, its output projection, pair-AllReduce chunk and
LayerNorm are issued, overlapping the collective with later q-tiles'
attention instead of paying a serial tail.
"""

import numpy as np

import concourse.bass as bass
import concourse.mybir as mybir
import concourse.tile as tile
from concourse import bacc
from concourse.bass_utils import run_bass_kernel_spmd

# Problem constants (hardcoded per harness contract)
B, T, C = 4, 2048, 1024
H, D = 16, 64
HG = 2                 # head groups (cores per batch)
HPG = H // HG          # heads per group = 8
CG = C // HG           # channels per group = 512
SCALE = D ** -0.5      # 0.125
LN_EPS = 1e-5

QT = 512               # q tile (moving free dim)
KT = 128               # k tile (PE contraction tile)
NQT = T // QT          # 4
NKC = T // KT          # 16
NIC = C // 128         # 8 input-channel chunks
NDC = CG // 128        # 4 output d-chunks per group

F32 = mybir.dt.float32
F32R = mybir.dt.float32r
BF16 = mybir.dt.bfloat16

REPLICA_GROUPS = [[0, 1], [2, 3], [4, 5], [6, 7]]


def build_program():
    """Build + compile the single-core SPMD Bass program. Returns (nc, io)."""
    nc = bacc.Bacc(
        "TRN2",
        target_bir_lowering=False,
        debug=False,
        enable_asserts=False,
        num_devices=8,
    )

    # ---- DRAM I/O ----  (f32r inputs are pre-rounded fp32 on the host)
    xT = nc.dram_tensor("xT", [C, T], F32R, kind="ExternalInput")
    wqT = nc.dram_tensor("wqT", [C, CG], F32R, kind="ExternalInput")
    wkT = nc.dram_tensor("wkT", [C, CG], F32R, kind="ExternalInput")
    wvT = nc.dram_tensor("wvT", [C, CG], F32R, kind="ExternalInput")
    wpT = nc.dram_tensor("wpT", [CG, C], F32R, kind="ExternalInput")
    gamma = nc.dram_tensor("gamma", [C], F32, kind="ExternalInput")
    beta = nc.dram_tensor("beta", [C], F32, kind="ExternalInput")
    # 4 diagonal-block masks [128 k x 512 q]: 1.0 where 128*j + k_r <= q_r
    masks = nc.dram_tensor("masks", [4, KT, QT], BF16, kind="ExternalInput")
    y_out = nc.dram_tensor("y", [T, C], F32, kind="ExternalOutput")

    with tile.TileContext(nc) as tc:
        _body(tc, xT, wqT, wkT, wvT, wpT, gamma, beta, masks, y_out)

    nc.compile()
    io = dict(inputs=["xT", "wqT", "wkT", "wvT", "wpT", "gamma", "beta", "masks"],
              output="y")
    return nc, io


def _body(tc, xT, wqT, wkT, wvT, wpT, gamma, beta, masks, y_out):
    nc = tc.nc

    # ---------- persistent SBUF ----------
    persist = tc.alloc_tile_pool(name="persist", bufs=1)
    # K^T / Q^T in [128 part, d-chunk, t] layout; head h lives at partition
    # rows 64*(h%2) .. +64 of chunk h//2.  Q^T is overwritten in-place by the
    # normalized attention output O^T (identical layout), which then feeds the
    # output projection as lhsT.
    kT_sb = persist.tile([128, NDC, T], F32R)
    qT_sb = persist.tile([128, NDC, T], F32R)
    # V in [t(128-chunks) part, k-chunk, head, 65] layout; col 64 is the ones
    # column providing the softmax denominator in the AV matmul.
    v_sb = persist.tile([128, NKC, HPG, 65], BF16)
    mask_sb = persist.tile([128, 4, QT], BF16)
    ones_sb = persist.tile([65, 64], F32R)
    eps_sb = persist.tile([128, 1], F32)

    nc.vector.memset(eps_sb, LN_EPS)
    # f32r/bf16 matmul operands cannot be memset directly; round via a copy
    ones_f = persist.tile([128, 128], F32)
    nc.vector.memset(ones_f, 1.0)
    nc.scalar.copy(ones_sb, ones_f[0:65, 0:64])
    # ones columns of V
    nc.scalar.copy(
        v_sb[:, :, :, 64],
        ones_f[:, 0:NKC * HPG].rearrange("p (a b) -> p a b", a=NKC),
    )

    # ---------- phase A: QKV projections ----------
    with (
        tc.tile_pool(name="wproj", bufs=1) as wpool,
        tc.tile_pool(name="xstream", bufs=2) as xpool,
        tc.tile_pool(name="psA", bufs=3, space="PSUM") as psA,
    ):
        wq_sb = wpool.tile([128, NIC, CG], F32R, tag="wq")
        wk_sb = wpool.tile([128, NIC, CG], F32R, tag="wk")
        wv_sb = wpool.tile([128, NIC, CG], F32R, tag="wv")
        # wk + first x tile first so K-projection matmuls start ASAP
        nc.sync.dma_start(out=wk_sb, in_=wkT.ap().rearrange("(a p) o -> p a o", p=128))
        x_t0 = xpool.tile([128, NIC, QT], F32R, name="x_t")
        nc.sync.dma_start(out=x_t0, in_=xT.ap()[:, 0:QT].rearrange("(a p) t -> p a t", p=128))
        nc.sync.dma_start(out=wq_sb, in_=wqT.ap().rearrange("(a p) o -> p a o", p=128))
        nc.sync.dma_start(out=wv_sb, in_=wvT.ap().rearrange("(a p) o -> p a o", p=128))
        nc.sync.dma_start(out=mask_sb, in_=masks.ap().rearrange("j k q -> k j q"))

        for tt in range(NQT):  # t tiles of 512
            ts = slice(tt * QT, (tt + 1) * QT)
            if tt == 0:
                x_t = x_t0
            else:
                x_t = xpool.tile([128, NIC, QT], F32R, name="x_t")
                nc.sync.dma_start(out=x_t, in_=xT.ap()[:, ts].rearrange("(a p) t -> p a t", p=128))

            # K^T and Q^T: out[d_chunk, t] = sum_i W[d, i] * xT[i, t]
            for w_sb, dst in ((wk_sb, kT_sb), (wq_sb, qT_sb)):
                for dc in range(NDC):
                    ps = psA.tile([128, QT], F32, tag="ps")
                    for ic in range(NIC):
                        nc.tensor.matmul(
                            ps,
                            w_sb[:, ic, dc * 128:(dc + 1) * 128],
                            x_t[:, ic, :],
                            start=(ic == 0), stop=(ic == NIC - 1),
                        )
                    nc.scalar.copy(dst[:, dc, ts], ps)

            # V: out[t(128) part, d] = sum_i xT[i, t] * WvT[i, d]
            for j in range(QT // 128):
                kc = tt * (QT // 128) + j
                ps = psA.tile([128, CG], F32, tag="ps")
                for ic in range(NIC):
                    nc.tensor.matmul(
                        ps,
                        x_t[:, ic, j * 128:(j + 1) * 128],
                        wv_sb[:, ic, :],
                        start=(ic == 0), stop=(ic == NIC - 1),
                    )
                nc.scalar.copy(
                    v_sb[:, kc, :, 0:64],
                    ps.rearrange("p (h d) -> p h d", h=HPG),
                )

    # ---------- phase B+C interleaved: attention, projection, AllReduce, LN ----------
    with (
        tc.tile_pool(name="psS", bufs=2, space="PSUM") as psS,
        tc.tile_pool(name="psO", bufs=3, space="PSUM") as psO,
        tc.tile_pool(name="psMisc", bufs=1, space="PSUM") as psM,
        tc.tile_pool(name="pT", bufs=4) as ppool,
        tc.tile_pool(name="norm", bufs=4) as npool,
        tc.tile_pool(name="wp", bufs=1) as wppool,
        tc.tile_pool(name="ytile", bufs=3) as ypool,
        tc.tile_pool(name="lnst", bufs=4) as lnpool,
        tc.tile_pool(name="dram", bufs=1, space="DRAM") as dram,
    ):
        wp_sb = wppool.tile([128, NDC, C], F32R)
        nc.sync.dma_start(out=wp_sb, in_=wpT.ap().rearrange("(a p) o -> p a o", p=128))
        gamma_sb = wppool.tile([128, C], F32)
        beta_sb = wppool.tile([128, C], F32)
        nc.gpsimd.dma_start(out=gamma_sb, in_=gamma.ap().unsqueeze(0).to_broadcast([128, C]))
        nc.gpsimd.dma_start(out=beta_sb, in_=beta.ap().unsqueeze(0).to_broadcast([128, C]))

        y_part = dram.tile([T, C], F32)
        y_red = dram.tile([T, C], F32)

        def attention(hp, qt):
            nkt = (qt + 1) * (QT // KT)  # causal k-extent in 128-tiles
            ps_o = [psO.tile([65, QT], F32, tag="pso", name=f"ps_o{e}")
                    for e in range(2)]

            def s_pair(kt):
                # S^T[k, q] = K[k, :] . Q[q, :]; heads 2hp / 2hp+1 live on
                # partition rows 0-63 / 64-127 -> row-tiled, run concurrent
                ps_s = psS.tile([128, 2, QT], F32, tag="pss", name="ps_s")
                for e in range(2):
                    r0, r1 = 64 * e, 64 * e + 64
                    nc.tensor.matmul(
                        ps_s[:, e, :],
                        kT_sb[r0:r1, hp, kt * KT:(kt + 1) * KT],
                        qT_sb[r0:r1, hp, qt * QT:(qt + 1) * QT],
                        start=True, stop=True,
                    )
                return ps_s

            # software-pipelined: S-pair for kt+1 issues before AV of kt
            ps_s_cur = s_pair(0)
            for kt in range(nkt):
                ps_s_next = s_pair(kt + 1) if kt + 1 < nkt else None
                diag = kt - (nkt - 4)  # >=0 on diagonal blocks
                p_t = ppool.tile([128, 2, QT], BF16, tag="pt")
                # exp over both heads' tiles in one ACT call
                nc.scalar.activation(
                    p_t, ps_s_cur, mybir.ActivationFunctionType.Exp, scale=SCALE,
                )
                if diag >= 0:
                    for e in range(2):
                        nc.vector.tensor_mul(
                            p_t[:, e, :], p_t[:, e, :], mask_sb[:, diag, :]
                        )
                for e in range(2):
                    h = 2 * hp + e
                    nc.tensor.matmul(
                        ps_o[e],
                        v_sb[:, kt, h, :],
                        p_t[:, e, :],
                        start=(kt == 0), stop=(kt == nkt - 1),
                    )
                ps_s_cur = ps_s_next

            # normalize: O^T[d, q] /= denom[q]; write into qT_sb (=out^T)
            for e in range(2):
                d_r = npool.tile([65, QT], F32R, tag="denr")
                nc.scalar.copy(d_r[64:65, :], ps_o[e][64:65, :])
                # broadcast denom across 64 partitions via ones outer-product
                db_ps = psM.tile([64, QT], F32, tag="misc", name="db_ps")
                nc.tensor.matmul(
                    db_ps, ones_sb[64:65, :], d_r[64:65, :],
                    start=True, stop=True,
                )
                rb = npool.tile([64, QT], F32, tag="rbcast")
                nc.vector.reciprocal_approx_fast(out=rb, in_=db_ps)
                if e == 0:
                    nc.vector.tensor_mul(
                        qT_sb[0:64, hp, qt * QT:(qt + 1) * QT],
                        ps_o[e][0:64, :], rb,
                    )
                else:
                    o_n = npool.tile([64, QT], F32R, tag="onorm")
                    nc.vector.tensor_mul(o_n, ps_o[e][0:64, :], rb)
                    nc.sync.dma_start(
                        out=qT_sb[64:128, hp, qt * QT:(qt + 1) * QT], in_=o_n
                    )

        def out_proj(qt):
            # y_part rows [512*qt, 512*qt+512) = out^T.T @ WpT  (partial sums)
            for tn in range(qt * (QT // 128), (qt + 1) * (QT // 128)):
                y_sb = ypool.tile([128, C], F32, tag="ysb")
                for ct in range(C // QT):
                    ps = psM.tile([128, QT], F32, tag="misc", name="ps_y")
                    for cc in range(NDC):
                        nc.tensor.matmul(
                            ps,
                            qT_sb[:, cc, tn * 128:(tn + 1) * 128],
                            wp_sb[:, cc, ct * QT:(ct + 1) * QT],
                            start=(cc == 0), stop=(cc == NDC - 1),
                        )
                    nc.vector.tensor_copy(y_sb[:, ct * QT:(ct + 1) * QT], ps)
                nc.sync.dma_start(out=y_part[tn * 128:(tn + 1) * 128, :], in_=y_sb)

        def layer_norm(qt):
            for tn in range(qt * (QT // 128), (qt + 1) * (QT // 128)):
                y_t = ypool.tile([128, C], F32, tag="yln")
                nc.sync.dma_start(out=y_t, in_=y_red[tn * 128:(tn + 1) * 128, :])
                stats = lnpool.tile([128, 2, 6], F32, tag="stats")
                mv = lnpool.tile([128, 2], F32, tag="mv")
                yv = y_t.rearrange("p (s f) -> p s f", s=2)
                for s in range(2):
                    nc.vector.bn_stats(out=stats[:, s, :], in_=yv[:, s, :])
                nc.vector.bn_aggr(out=mv, in_=stats)
                rstd = lnpool.tile([128, 1], F32, tag="rstd")
                nc.scalar.activation(
                    out=rstd, in_=mv[:, 1:2],
                    func=mybir.ActivationFunctionType.Sqrt,
                    bias=eps_sb, scale=1.0,
                )
                nc.vector.reciprocal(rstd, rstd)
                nc.vector.tensor_scalar(
                    out=y_t, in0=y_t,
                    scalar1=mv[:, 0:1], scalar2=rstd,
                    op0=mybir.AluOpType.subtract, op1=mybir.AluOpType.mult,
                )
                nc.vector.tensor_mul(y_t, y_t, gamma_sb)
                nc.vector.tensor_add(y_t, y_t, beta_sb)
                nc.sync.dma_start(out=y_out.ap()[tn * 128:(tn + 1) * 128, :], in_=y_t)

        # q-tile-major: attention for all head pairs of this q-tile, then its
        # output projection + AllReduce chunk + LayerNorm, which overlap the
        # next q-tile's attention.
        for qt in range(NQT):
            for hp in range(HPG // 2):
                attention(hp, qt)
            out_proj(qt)
            rs = slice(qt * QT, (qt + 1) * QT)
            nc.gpsimd.collective_compute(
                "AllReduce",
                mybir.AluOpType.add,
                replica_groups=REPLICA_GROUPS,
                ins=[y_part[rs, :]],
                outs=[y_red[rs, :]],
            )
            layer_norm(qt)

    persist.release()


_PROG = None


def _get_program():
    global _PROG
    if _PROG is None:
        _PROG = build_program()
    return _PROG


def _round_f32r(a):
    """Round fp32 to the f32r grid (11 explicit mantissa bits, RNE-ish)."""
    bits = np.ascontiguousarray(a, np.float32).view(np.uint32)
    r = ((bits.astype(np.uint64) + 0x800) & 0xFFFFF000).astype(np.uint32)
    return r.view(np.float32)


def make_in_maps(x, Wk, Wq, Wv, Wp, gamma, beta):
    import ml_dtypes
    x = np.asarray(x, dtype=np.float32)
    masks = np.zeros((4, KT, QT), dtype=np.float32)
    for j in range(4):
        k = np.arange(KT)[:, None]
        q = np.arange(QT)[None, :]
        masks[j] = (128 * j + k <= q).astype(np.float32)
    masks = masks.astype(ml_dtypes.bfloat16)
    in_maps = []
    for c in range(8):
        b, hg = c // HG, c % HG
        sl = slice(hg * CG, (hg + 1) * CG)
        in_maps.append({
            "xT": _round_f32r(x[b].T),
            "wqT": _round_f32r(np.asarray(Wq, np.float32)[sl, :].T),
            "wkT": _round_f32r(np.asarray(Wk, np.float32)[sl, :].T),
            "wvT": _round_f32r(np.asarray(Wv, np.float32)[sl, :].T),
            "wpT": _round_f32r(np.asarray(Wp, np.float32)[:, sl].T),
            "gamma": np.asarray(gamma, np.float32),
            "beta": np.asarray(beta, np.float32),
            "masks": masks,
        })
    return in_maps


def kernel(x, Wk, Wq, Wv, Wp, gamma, beta, _trace=False, _trace_kwargs=None):
    nc, io = _get_program()
    in_maps = make_in_maps(x, Wk, Wq, Wv, Wp, gamma, beta)
    res = run_bass_kernel_spmd(
        nc, in_maps, core_ids=list(range(8)),
        trace=_trace, **(_trace_kwargs or {}),
    )
    out = np.stack([res.results[HG * b]["y"] for b in range(B)])
    if _trace:
        kernel.last_results = res
    return out
